# revision 27
# baseline (speedup 1.0000x reference)
"""GRU decoder kernel for Trainium2 (8 NeuronCores, data-parallel over batch).

Problem (hardcoded): B=4096, T=168, D=64, H=128.
  per step t:  gx_t = feats_t @ W_ih[:, :D].T + b_ih + y_prev * w_y
               gh   = h @ W_hh.T + b_hh
               r = sig(gx_r+gh_r); z = sig(gx_z+gh_z)
               n = tanh(gx_n + r*gh_n)
               h = (1-z)*n + z*h;  y = h @ wo + bo

Mapping per core (batch shard BS=512), layout [gate/hidden dim on
partitions, batch on free]:
  - Host pre-casts feats/h0/weights to fp16. feats stream in natural
    [b, (t,d)] layout via plain chunked DMAs, then PE transpose-mode flips
    each [128b, 128] block into [d, b] tiles (2 timesteps stacked on
    partitions: even t rows 0-63, odd rows 64-127); one ACT copy per t-pair
    evacuates PSUM -> SBUF.
  - For t>=1 the y-feedback is folded into the hidden matmuls:
      r/z:  W1 = W_hh[rz] + w_y[rz] (x) wo
      n:    rank-1 matmul (wo (x) w_y_n) @ h accumulated onto the gx_n psum
    (valid because y_prev = wo . h_prev + bo; bo is folded into biases).
    Step 0 uses the supplied y0 via K=1 matmuls.
  - Gate math: sigmoids + tanh on ACT (psum-source, per-partition biases
    ride free), the two n-gate combines as DVE scalar_tensor_tensor,
    h' = n + z*(h-n) split DVE/GpSimd. All gate tensors fp16.
  - y_t = wo . h_t via an M=32 matmul into psum col-group t%4; every 4 steps
    one DVE op (+bo, ->fp16) appends to an SBUF accumulator, DMA'd out once
    at the end (output [T, BS] fp16; host transposes + casts to fp32).
"""

import numpy as np

import concourse.bacc as bacc
import concourse.bass as bass
import concourse.mybir as mybir
import concourse.tile as tile
from concourse.bass_utils import run_bass_kernel_spmd

B, T, D, H = 4096, 168, 64, 128
NCORES = 8
BS = B // NCORES  # 512

F32 = mybir.dt.float32
F16 = mybir.dt.float16
AF = mybir.ActivationFunctionType
ALU = mybir.AluOpType

TC = 28     # timesteps per feats DMA chunk
NSLOT = 8   # featsT ring slots (t-pairs)

# packed-constants fp16 column layout
_WOC0, _WYR0, _Y00, _B0, _ID0 = 0, 32, 416, 928, 944
NPACK = 944 + 16 + 128


def build(nt=T):
    """Build the per-core Bass program. nt: number of timesteps (tests)."""
    assert nt % 4 == 0
    nc = bacc.Bacc("TRN2", target_bir_lowering=False, debug=False)

    feats = nc.declare_dram_parameter("feats", [BS, nt, D], F16, isOutput=False)
    h0 = nc.declare_dram_parameter("h0", [BS, H], F16, isOutput=False)
    wft_d = nc.declare_dram_parameter("wft", [128, 384], F16, isOutput=False)
    w1t_d = nc.declare_dram_parameter("w1t", [128, 256], F16, isOutput=False)
    whhnt_d = nc.declare_dram_parameter("whhnt", [128, 128], F16, isOutput=False)
    wynt_d = nc.declare_dram_parameter("wynt", [128, 128], F16, isOutput=False)
    whhrz0_d = nc.declare_dram_parameter("whhrz0", [128, 256], F16, isOutput=False)
    pack = nc.declare_dram_parameter("pack", [128, NPACK], F16, isOutput=False)

    yT = nc.declare_dram_parameter("yT", [nt, BS], F16, isOutput=True)
    nflush = nt // 4
    nchunk = (nt + TC - 1) // TC

    with tile.TileContext(nc) as tc:
        with (
            tc.tile_pool(name="wpool", bufs=1) as wpool,
            tc.tile_pool(name="fraw", bufs=2) as fraw_pool,
            tc.tile_pool(name="ftr", bufs=NSLOT) as ftr_pool,
            tc.tile_pool(name="hpool", bufs=2) as hpool,
            tc.tile_pool(name="gpool", bufs=2) as gpool,
            tc.tile_pool(name="ps_rz", bufs=2, space="PSUM") as ps_rz,
            tc.tile_pool(name="ps_gx", bufs=1, space="PSUM") as ps_gx,
            tc.tile_pool(name="ps_gh", bufs=1, space="PSUM") as ps_gh,
            tc.tile_pool(name="ps_u", bufs=1, space="PSUM") as ps_u,
            tc.tile_pool(name="ps_tr", bufs=1, space="PSUM") as ps_tr,
        ):
            # ---- constants ----
            pk = wpool.tile([128, NPACK], F16)
            nc.sync.dma_start(pk[:], pack[:])
            woc = pk[:, _WOC0:_WOC0 + 32]
            wyrow = pk[0:1, _WYR0:_WYR0 + 384]
            y0_sb = pk[0:1, _Y00:_Y00 + 512]
            brz1 = pk[:, _B0:_B0 + 4].bitcast(F32)
            brz0 = pk[:, _B0 + 4:_B0 + 8].bitcast(F32)
            bn1 = pk[:, _B0 + 8:_B0 + 10].bitcast(F32)
            bn0 = pk[:, _B0 + 10:_B0 + 12].bitcast(F32)
            bhn = pk[:, _B0 + 12:_B0 + 14].bitcast(F32)
            bo_t = pk[:, _B0 + 14:_B0 + 16].bitcast(F32)
            ident = pk[:, _ID0:_ID0 + 128]

            wft = wpool.tile([128, 384], F16)
            w1t = wpool.tile([128, 256], F16)
            whhnt = wpool.tile([128, 128], F16)
            wynt = wpool.tile([128, 128], F16)
            whhrz0 = wpool.tile([128, 256], F16)
            for sb, dr in [
                (wft, wft_d), (w1t, w1t_d), (whhnt, whhnt_d),
                (wynt, wynt_d), (whhrz0, whhrz0_d),
            ]:
                nc.sync.dma_start(sb[:], dr[:])

            # ---- h0 -> hT via PE transpose ----
            h0st = wpool.tile([128, 512], F16)
            for j in range(4):
                nc.sync.dma_start(
                    h0st[:, j * 128:(j + 1) * 128], h0[j * 128:(j + 1) * 128, :],
                )
            p_t0 = ps_tr.tile([128, BS], F16, tag="p_tr")
            for j in range(4):
                nc.tensor.transpose(
                    p_t0[:, j * 128:(j + 1) * 128], h0st[:, j * 128:(j + 1) * 128],
                    ident,
                )
            hT_prev = hpool.tile([128, BS], F16, tag="hT")
            nc.scalar.copy(hT_prev[:], p_t0[:])

            # ---- feats pipeline ----
            ftr_tiles = {}
            raw_tiles = {}

            def load_chunk(c):
                t0 = c * TC
                tcn = min(TC, nt - t0)
                raws = []
                for j in range(4):
                    raw = fraw_pool.tile([128, TC * 64], F16, tag=f"fraw{j}")
                    nc.sync.dma_start(
                        raw[:, : tcn * 64],
                        feats[j * 128:(j + 1) * 128, t0:t0 + tcn, :],
                    )
                    raws.append(raw)
                raw_tiles[c] = raws

            def transpose_pair(p):
                c, po = divmod(p, TC // 2)
                raws = raw_tiles[c]
                p_tr = ps_tr.tile([128, BS], F16, tag="p_tr")
                for j in range(4):
                    nc.tensor.transpose(
                        p_tr[:, j * 128:(j + 1) * 128],
                        raws[j][:, po * 128:(po + 1) * 128], ident,
                    )
                ftile = ftr_pool.tile([128, BS], F16, tag="ftr")
                nc.scalar.copy(ftile[:], p_tr[:])
                ftr_tiles[p] = ftile

            load_chunk(0)
            yacc = wpool.tile([128, nflush * 512], F16)

            # ---- recurrence ----
            for t in range(nt):
                if t % TC == 0 and t // TC + 1 < nchunk:
                    load_chunk(t // TC + 1)
                if t % 2 == 0:
                    # stay a few pairs ahead of the consumer
                    want = min(t // 2 + 2, nt // 2 - 1)
                    for p in range(len(ftr_tiles), want + 1):
                        transpose_pair(p)

                ftile = ftr_tiles[t // 2]
                half = (t % 2) * 64
                rhs_f = ftile[half:half + 64, :]
                wf = wft[half:half + 64, :]

                p_r = ps_rz.tile([128, BS], F32, tag="p_r")
                p_z = ps_rz.tile([128, BS], F32, tag="p_z")
                p_gx = ps_gx.tile([128, BS], F32, tag="p_gx")
                p_gh = ps_gh.tile([128, BS], F32, tag="p_gh")

                # feats matmuls (independent of h)
                nc.tensor.matmul(p_r[:], wf[:, 0:128], rhs_f, start=True, stop=False)
                nc.tensor.matmul(p_z[:], wf[:, 128:256], rhs_f, start=True, stop=False)
                nc.tensor.matmul(p_gx[:], wf[:, 256:384], rhs_f, start=True, stop=False)

                # hidden matmuls
                if t == 0:
                    nc.tensor.matmul(p_r[:], whhrz0[:, 0:128], hT_prev[:], start=False, stop=False)
                    nc.tensor.matmul(p_z[:], whhrz0[:, 128:256], hT_prev[:], start=False, stop=False)
                    nc.tensor.matmul(p_r[:], wyrow[0:1, 0:128], y0_sb, start=False, stop=True)
                    nc.tensor.matmul(p_z[:], wyrow[0:1, 128:256], y0_sb, start=False, stop=True)
                    nc.tensor.matmul(p_gx[:], wyrow[0:1, 256:384], y0_sb, start=False, stop=True)
                else:
                    nc.tensor.matmul(p_r[:], w1t[:, 0:128], hT_prev[:], start=False, stop=True)
                    nc.tensor.matmul(p_z[:], w1t[:, 128:256], hT_prev[:], start=False, stop=True)
                    nc.tensor.matmul(p_gx[:], wynt[:], hT_prev[:], start=False, stop=True)
                nc.tensor.matmul(p_gh[:], whhnt[:], hT_prev[:], start=True, stop=True)

                brz = brz0 if t == 0 else brz1
                bn = bn0 if t == 0 else bn1

                r16 = gpool.tile([128, BS], F16, tag="r16")
                z16 = gpool.tile([128, BS], F16, tag="z16")
                nc.scalar.activation(r16[:], p_r[:], AF.Sigmoid, bias=brz[:, 0:1])
                nc.scalar.activation(z16[:], p_z[:], AF.Sigmoid, bias=brz[:, 1:2])

                t1 = gpool.tile([128, BS], F16, tag="t1")
                npre = gpool.tile([128, BS], F16, tag="npre")
                n16 = gpool.tile([128, BS], F16, tag="n16")
                # t1 = (gh_n + b_hh_n) * r ; npre = (gx_n + b_n) + t1
                nc.vector.scalar_tensor_tensor(t1[:], p_gh[:], bhn[:, 0:1], r16[:], ALU.add, ALU.mult)
                nc.vector.scalar_tensor_tensor(npre[:], p_gx[:], bn[:, 0:1], t1[:], ALU.add, ALU.add)
                nc.scalar.activation(n16[:], npre[:], AF.Tanh)

                hmn = gpool.tile([128, BS], F16, tag="hmn")
                zh = gpool.tile([128, BS], F16, tag="zh")
                hT_cur = hpool.tile([128, BS], F16, tag="hT")
                nc.gpsimd.tensor_tensor(hmn[:], hT_prev[:], n16[:], ALU.subtract)
                nc.vector.tensor_tensor(zh[:], z16[:], hmn[:], ALU.mult)
                nc.vector.tensor_tensor(hT_cur[:], n16[:], zh[:], ALU.add)

                # y_t = wo . h_t (+bo), 32 duplicated rows per psum col-group
                c4 = t % 4
                if c4 == 0:
                    p_u = ps_u.tile([128, BS], F32, tag="p_u")
                nc.tensor.matmul(
                    p_u[32 * c4:32 * (c4 + 1), :], woc, hT_cur[:],
                    start=True, stop=True, tile_position=(0, 32 * c4),
                )
                if c4 == 3:
                    g = t // 4
                    nc.vector.tensor_scalar(
                        yacc[:, g * 512:(g + 1) * 512], p_u[:], bo_t[:, 0:1],
                        None, ALU.add,
                    )

                hT_prev = hT_cur

            # ---- single y writeback ----
            nc.sync.dma_start(
                yT.rearrange("(g c) b -> c g b", c=4),
                yacc[0:128:32, :].rearrange("p (g b) -> p g b", b=512),
            )

    nc.compile()
    return nc


# -------- host-side weight prep + sharded execution --------

def _prep_aux(W_ih, W_hh, b_ih, b_hh, Wo, bo):
    W_ih = np.asarray(W_ih, np.float32)
    W_hh = np.asarray(W_hh, np.float32)
    b_ih = np.asarray(b_ih, np.float32)
    b_hh = np.asarray(b_hh, np.float32)
    wo = np.asarray(Wo, np.float32)[0]       # [H]
    bo_s = float(np.asarray(bo, np.float32)[0])
    wfd = W_ih[:, :D]                         # [3H, D]
    w_y = W_ih[:, D]                          # [3H]

    wft = np.zeros((128, 384), np.float16)
    wft[0:64] = wfd.T.astype(np.float16)
    wft[64:128] = wfd.T.astype(np.float16)

    W1 = W_hh[0:2 * H] + np.outer(w_y[0:2 * H], wo)       # [2H, H]
    aux = dict(
        wft=wft,
        w1t=np.ascontiguousarray(W1.T.astype(np.float16)),
        whhnt=np.ascontiguousarray(W_hh[2 * H:].T.astype(np.float16)),
        wynt=np.ascontiguousarray(np.outer(wo, w_y[2 * H:]).astype(np.float16)),
        whhrz0=np.ascontiguousarray(W_hh[0:2 * H].T.astype(np.float16)),
    )

    pk = np.zeros((128, NPACK), np.float16)
    pk[:, _WOC0:_WOC0 + 32] = np.repeat(wo[:, None], 32, axis=1).astype(np.float16)
    pk[0, _WYR0:_WYR0 + 384] = w_y.astype(np.float16)
    brz_base = (b_ih + b_hh)[0:2 * H]
    brz1 = np.stack(
        [brz_base[0:H] + w_y[0:H] * bo_s, brz_base[H:2 * H] + w_y[H:2 * H] * bo_s],
        axis=1,
    ).astype(np.float32)
    brz0 = np.stack([brz_base[0:H], brz_base[H:2 * H]], axis=1).astype(np.float32)
    bn1 = (b_ih[2 * H:] + w_y[2 * H:] * bo_s)[:, None].astype(np.float32)
    bn0 = b_ih[2 * H:][:, None].astype(np.float32)
    bhn = b_hh[2 * H:][:, None].astype(np.float32)
    bo_a = np.full((128, 1), bo_s, np.float32)
    block = np.concatenate([brz1, brz0, bn1, bn0, bhn, bo_a], axis=1)  # [128, 8]
    pk[:, _B0:_B0 + 16] = np.ascontiguousarray(block).view(np.float16)
    pk[:, _ID0:_ID0 + 128] = np.eye(128, dtype=np.float16)
    aux["pack"] = pk
    return aux


_NC_CACHE = {}


def kernel(future_feats, h0, y0, W_ih, W_hh, b_ih, b_hh, Wo, bo):
    future_feats = np.ascontiguousarray(np.asarray(future_feats).astype(np.float16))
    h0f = np.ascontiguousarray(np.asarray(h0).astype(np.float16)[0])   # [B, H]
    y0f = np.asarray(y0).astype(np.float16)                            # [B]

    aux = _prep_aux(W_ih, W_hh, b_ih, b_hh, Wo, bo)

    if "nc" not in _NC_CACHE:
        _NC_CACHE["nc"] = build(T)
    nc = _NC_CACHE["nc"]

    in_maps = []
    for c in range(NCORES):
        sl = slice(c * BS, (c + 1) * BS)
        m = dict(aux)
        pk = aux["pack"].copy()
        pk[0, _Y00:_Y00 + 512] = y0f[sl]
        m["pack"] = pk
        m["feats"] = future_feats[sl]
        m["h0"] = h0f[sl]
        in_maps.append(m)

    res = run_bass_kernel_spmd(nc, in_maps, core_ids=list(range(NCORES)))
    outs = [r["yT"] for r in res.results]
    return np.concatenate([o.T.astype(np.float32) for o in outs], axis=0)


# revision 29
# speedup vs baseline: 1.2439x; 1.2439x over previous
"""GRU decoder kernel for Trainium2 (8 NeuronCores, data-parallel over batch).

Problem (hardcoded): B=4096, T=168, D=64, H=128.
  per step t:  gx_t = feats_t @ W_ih[:, :D].T + b_ih + y_prev * w_y
               gh   = h @ W_hh.T + b_hh
               r = sig(gx_r+gh_r); z = sig(gx_z+gh_z)
               n = tanh(gx_n + r*gh_n)
               h = (1-z)*n + z*h;  y = h @ wo + bo

Mapping per core (batch shard BS=512), layout [gate/hidden dim on
partitions, batch on free]:
  - Host pre-casts feats/h0/weights to fp16. feats stream in natural
    [b, (t,d)] layout via plain chunked DMAs, then PE transpose-mode flips
    each [128b, 128] block into [d, b] tiles (2 timesteps stacked on
    partitions: even t rows 0-63, odd rows 64-127); one ACT copy per t-pair
    evacuates PSUM -> SBUF.
  - For t>=1 the y-feedback is folded into the hidden matmuls:
      r/z:  W1 = W_hh[rz] + w_y[rz] (x) wo
      n:    rank-1 matmul (wo (x) w_y_n) @ h accumulated onto the gx_n psum
    (valid because y_prev = wo . h_prev + bo; bo is folded into biases).
    Step 0 uses the supplied y0 via K=1 matmuls.
  - Gate math: sigmoids + tanh on ACT (psum-source, per-partition biases
    ride free), the two n-gate combines as DVE scalar_tensor_tensor,
    h' = n + z*(h-n) split DVE/GpSimd. All gate tensors fp16.
  - y_t = wo . h_t via an M=32 matmul into psum col-group t%4; every 4 steps
    one DVE op (+bo, ->fp16) appends to an SBUF accumulator, DMA'd out once
    at the end (output [T, BS] fp16; host transposes + casts to fp32).
"""

import numpy as np

import concourse.bacc as bacc
import concourse.bass as bass
import concourse.mybir as mybir
import concourse.tile as tile
from concourse.bass_utils import run_bass_kernel_spmd

B, T, D, H = 4096, 168, 64, 128
NCORES = 8
BS = B // NCORES  # 512

F32 = mybir.dt.float32
F16 = mybir.dt.float16
AF = mybir.ActivationFunctionType
ALU = mybir.AluOpType

TC = 28     # timesteps per feats DMA chunk
NSLOT = 8   # featsT ring slots (t-pairs)

# packed-constants fp16 column layout
_WOC0, _WYR0, _Y00, _B0, _ID0 = 0, 32, 416, 928, 944
NPACK = 944 + 16 + 128


def build(nt=T):
    """Build the per-core Bass program. nt: number of timesteps (tests)."""
    assert nt % 4 == 0
    nc = bacc.Bacc("TRN2", target_bir_lowering=False, debug=False)

    feats = nc.declare_dram_parameter("feats", [BS, nt, D], F16, isOutput=False)
    h0 = nc.declare_dram_parameter("h0", [BS, H], F16, isOutput=False)
    wft_d = nc.declare_dram_parameter("wft", [128, 384], F16, isOutput=False)
    w1t_d = nc.declare_dram_parameter("w1t", [128, 256], F16, isOutput=False)
    whhnt_d = nc.declare_dram_parameter("whhnt", [128, 128], F16, isOutput=False)
    wynt_d = nc.declare_dram_parameter("wynt", [128, 128], F16, isOutput=False)
    whhrz0_d = nc.declare_dram_parameter("whhrz0", [128, 256], F16, isOutput=False)
    pack = nc.declare_dram_parameter("pack", [128, NPACK], F16, isOutput=False)

    yT = nc.declare_dram_parameter("yT", [nt, BS], F16, isOutput=True)
    nflush = nt // 4
    nchunk = (nt + TC - 1) // TC

    with tile.TileContext(nc) as tc:
        with (
            tc.tile_pool(name="wpool", bufs=1) as wpool,
            tc.tile_pool(name="fraw", bufs=2) as fraw_pool,
            tc.tile_pool(name="ftr", bufs=NSLOT) as ftr_pool,
            tc.tile_pool(name="hpool", bufs=2) as hpool,
            tc.tile_pool(name="gpool", bufs=2) as gpool,
            tc.tile_pool(name="ps_rz", bufs=2, space="PSUM") as ps_rz,
            tc.tile_pool(name="ps_gx", bufs=1, space="PSUM") as ps_gx,
            tc.tile_pool(name="ps_gh", bufs=1, space="PSUM") as ps_gh,
            tc.tile_pool(name="ps_u", bufs=1, space="PSUM") as ps_u,
            tc.tile_pool(name="ps_tr", bufs=1, space="PSUM") as ps_tr,
        ):
            # ---- constants ----
            pk = wpool.tile([128, NPACK], F16)
            nc.sync.dma_start(pk[:], pack[:])
            woc = pk[:, _WOC0:_WOC0 + 32]
            wyrow = pk[0:1, _WYR0:_WYR0 + 384]
            y0_sb = pk[0:1, _Y00:_Y00 + 512]
            brz1 = pk[:, _B0:_B0 + 4].bitcast(F32)
            brz0 = pk[:, _B0 + 4:_B0 + 8].bitcast(F32)
            bn1 = pk[:, _B0 + 8:_B0 + 10].bitcast(F32)
            bn0 = pk[:, _B0 + 10:_B0 + 12].bitcast(F32)
            bhn = pk[:, _B0 + 12:_B0 + 14].bitcast(F32)
            bo_t = pk[:, _B0 + 14:_B0 + 16].bitcast(F32)
            ident = pk[:, _ID0:_ID0 + 128]

            wft = wpool.tile([128, 384], F16)
            w1t = wpool.tile([128, 256], F16)
            whhnt = wpool.tile([128, 128], F16)
            wynt = wpool.tile([128, 128], F16)
            whhrz0 = wpool.tile([128, 256], F16)
            for sb, dr in [
                (wft, wft_d), (w1t, w1t_d), (whhnt, whhnt_d),
                (wynt, wynt_d), (whhrz0, whhrz0_d),
            ]:
                nc.sync.dma_start(sb[:], dr[:])

            # ---- h0 -> hT via PE transpose ----
            h0st = wpool.tile([128, 512], F16)
            for j in range(4):
                nc.sync.dma_start(
                    h0st[:, j * 128:(j + 1) * 128], h0[j * 128:(j + 1) * 128, :],
                )
            p_t0 = ps_tr.tile([128, BS], F16, tag="p_tr")
            for j in range(4):
                nc.tensor.transpose(
                    p_t0[:, j * 128:(j + 1) * 128], h0st[:, j * 128:(j + 1) * 128],
                    ident,
                )
            hT_prev = hpool.tile([128, BS], F16, tag="hT")
            nc.scalar.copy(hT_prev[:], p_t0[:])

            # ---- feats pipeline ----
            ftr_tiles = {}
            raw_tiles = {}

            def load_chunk(c):
                t0 = c * TC
                tcn = min(TC, nt - t0)
                raws = []
                for j in range(4):
                    raw = fraw_pool.tile([128, TC * 64], F16, tag=f"fraw{j}")
                    nc.sync.dma_start(
                        raw[:, : tcn * 64],
                        feats[j * 128:(j + 1) * 128, t0:t0 + tcn, :],
                    )
                    raws.append(raw)
                raw_tiles[c] = raws

            def transpose_pair(p):
                c, po = divmod(p, TC // 2)
                raws = raw_tiles[c]
                p_tr = ps_tr.tile([128, BS], F16, tag="p_tr")
                for j in range(4):
                    nc.tensor.transpose(
                        p_tr[:, j * 128:(j + 1) * 128],
                        raws[j][:, po * 128:(po + 1) * 128], ident,
                    )
                ftile = ftr_pool.tile([128, BS], F16, tag="ftr")
                nc.scalar.copy(ftile[:], p_tr[:])
                ftr_tiles[p] = ftile

            load_chunk(0)
            yacc = wpool.tile([128, nflush * 512], F16)

            # ---- recurrence ----
            # feats matmuls are emitted one step ahead of the serial chain so
            # they fill PE gaps while the previous step's gate math runs.
            psums = {}

            def alloc_and_feats(t):
                ftile = ftr_tiles[t // 2]
                half = (t % 2) * 64
                rhs_f = ftile[half:half + 64, :]
                wf = wft[half:half + 64, :]
                p_r = ps_rz.tile([128, BS], F32, tag="p_r")
                p_z = ps_rz.tile([128, BS], F32, tag="p_z")
                p_gx = ps_gx.tile([128, BS], F32, tag="p_gx")
                nc.tensor.matmul(p_r[:], wf[:, 0:128], rhs_f, start=True, stop=False)
                nc.tensor.matmul(p_z[:], wf[:, 128:256], rhs_f, start=True, stop=False)
                nc.tensor.matmul(p_gx[:], wf[:, 256:384], rhs_f, start=True, stop=False)
                psums[t] = (p_r, p_z, p_gx)

            for t in range(nt):
                if t % TC == 0 and t // TC + 1 < nchunk:
                    load_chunk(t // TC + 1)
                if t % 2 == 0:
                    # stay a few pairs ahead of the consumer
                    want = min(t // 2 + 2, nt // 2 - 1)
                    for p in range(len(ftr_tiles), want + 1):
                        transpose_pair(p)
                if t == 0:
                    alloc_and_feats(0)

                p_r, p_z, p_gx = psums.pop(t)
                p_gh = ps_gh.tile([128, BS], F32, tag="p_gh")

                # hidden matmuls — r-path and gh first (they feed the chain)
                if t == 0:
                    nc.tensor.matmul(p_r[:], whhrz0[:, 0:128], hT_prev[:], start=False, stop=False)
                    nc.tensor.matmul(p_r[:], wyrow[0:1, 0:128], y0_sb, start=False, stop=True)
                    nc.tensor.matmul(p_gh[:], whhnt[:], hT_prev[:], start=True, stop=True)
                    nc.tensor.matmul(p_gx[:], wyrow[0:1, 256:384], y0_sb, start=False, stop=True)
                    nc.tensor.matmul(p_z[:], whhrz0[:, 128:256], hT_prev[:], start=False, stop=False)
                    nc.tensor.matmul(p_z[:], wyrow[0:1, 128:256], y0_sb, start=False, stop=True)
                else:
                    nc.tensor.matmul(p_r[:], w1t[:, 0:128], hT_prev[:], start=False, stop=True)
                    nc.tensor.matmul(p_gh[:], whhnt[:], hT_prev[:], start=True, stop=True)
                    nc.tensor.matmul(p_gx[:], wynt[:], hT_prev[:], start=False, stop=True)
                    nc.tensor.matmul(p_z[:], w1t[:, 128:256], hT_prev[:], start=False, stop=True)
                if t + 1 < nt:
                    alloc_and_feats(t + 1)

                brz = brz0 if t == 0 else brz1
                bn = bn0 if t == 0 else bn1

                r16 = gpool.tile([128, BS], F16, tag="r16")
                z16 = gpool.tile([128, BS], F16, tag="z16")
                nc.scalar.activation(r16[:], p_r[:], AF.Sigmoid, bias=brz[:, 0:1])
                nc.scalar.activation(z16[:], p_z[:], AF.Sigmoid, bias=brz[:, 1:2])

                t1 = gpool.tile([128, BS], F16, tag="t1")
                npre = gpool.tile([128, BS], F16, tag="npre")
                n16 = gpool.tile([128, BS], F16, tag="n16")
                # t1 = (gh_n + b_hh_n) * r ; npre = (gx_n + b_n) + t1
                nc.vector.scalar_tensor_tensor(t1[:], p_gh[:], bhn[:, 0:1], r16[:], ALU.add, ALU.mult)
                nc.vector.scalar_tensor_tensor(npre[:], p_gx[:], bn[:, 0:1], t1[:], ALU.add, ALU.add)
                nc.scalar.activation(n16[:], npre[:], AF.Tanh)

                hmn = gpool.tile([128, BS], F16, tag="hmn")
                zh = gpool.tile([128, BS], F16, tag="zh")
                hT_cur = hpool.tile([128, BS], F16, tag="hT")
                nc.vector.tensor_tensor(hmn[:], hT_prev[:], n16[:], ALU.subtract)
                nc.vector.tensor_tensor(zh[:], z16[:], hmn[:], ALU.mult)
                nc.vector.tensor_tensor(hT_cur[:], n16[:], zh[:], ALU.add)

                # y_t = wo . h_t (+bo), 32 duplicated rows per psum col-group
                c4 = t % 4
                if c4 == 0:
                    p_u = ps_u.tile([128, BS], F32, tag="p_u")
                nc.tensor.matmul(
                    p_u[32 * c4:32 * (c4 + 1), :], woc, hT_cur[:],
                    start=True, stop=True, tile_position=(0, 32 * c4),
                )
                if c4 == 3:
                    g = t // 4
                    nc.vector.tensor_scalar(
                        yacc[:, g * 512:(g + 1) * 512], p_u[:], bo_t[:, 0:1],
                        None, ALU.add,
                    )

                hT_prev = hT_cur

            # ---- single y writeback ----
            nc.sync.dma_start(
                yT.rearrange("(g c) b -> c g b", c=4),
                yacc[0:128:32, :].rearrange("p (g b) -> p g b", b=512),
            )

    nc.compile()
    return nc


# -------- host-side weight prep + sharded execution --------

def _prep_aux(W_ih, W_hh, b_ih, b_hh, Wo, bo):
    W_ih = np.asarray(W_ih, np.float32)
    W_hh = np.asarray(W_hh, np.float32)
    b_ih = np.asarray(b_ih, np.float32)
    b_hh = np.asarray(b_hh, np.float32)
    wo = np.asarray(Wo, np.float32)[0]       # [H]
    bo_s = float(np.asarray(bo, np.float32)[0])
    wfd = W_ih[:, :D]                         # [3H, D]
    w_y = W_ih[:, D]                          # [3H]

    wft = np.zeros((128, 384), np.float16)
    wft[0:64] = wfd.T.astype(np.float16)
    wft[64:128] = wfd.T.astype(np.float16)

    W1 = W_hh[0:2 * H] + np.outer(w_y[0:2 * H], wo)       # [2H, H]
    aux = dict(
        wft=wft,
        w1t=np.ascontiguousarray(W1.T.astype(np.float16)),
        whhnt=np.ascontiguousarray(W_hh[2 * H:].T.astype(np.float16)),
        wynt=np.ascontiguousarray(np.outer(wo, w_y[2 * H:]).astype(np.float16)),
        whhrz0=np.ascontiguousarray(W_hh[0:2 * H].T.astype(np.float16)),
    )

    pk = np.zeros((128, NPACK), np.float16)
    pk[:, _WOC0:_WOC0 + 32] = np.repeat(wo[:, None], 32, axis=1).astype(np.float16)
    pk[0, _WYR0:_WYR0 + 384] = w_y.astype(np.float16)
    brz_base = (b_ih + b_hh)[0:2 * H]
    brz1 = np.stack(
        [brz_base[0:H] + w_y[0:H] * bo_s, brz_base[H:2 * H] + w_y[H:2 * H] * bo_s],
        axis=1,
    ).astype(np.float32)
    brz0 = np.stack([brz_base[0:H], brz_base[H:2 * H]], axis=1).astype(np.float32)
    bn1 = (b_ih[2 * H:] + w_y[2 * H:] * bo_s)[:, None].astype(np.float32)
    bn0 = b_ih[2 * H:][:, None].astype(np.float32)
    bhn = b_hh[2 * H:][:, None].astype(np.float32)
    bo_a = np.full((128, 1), bo_s, np.float32)
    block = np.concatenate([brz1, brz0, bn1, bn0, bhn, bo_a], axis=1)  # [128, 8]
    pk[:, _B0:_B0 + 16] = np.ascontiguousarray(block).view(np.float16)
    pk[:, _ID0:_ID0 + 128] = np.eye(128, dtype=np.float16)
    aux["pack"] = pk
    return aux


_NC_CACHE = {}


def kernel(future_feats, h0, y0, W_ih, W_hh, b_ih, b_hh, Wo, bo):
    future_feats = np.ascontiguousarray(np.asarray(future_feats).astype(np.float16))
    h0f = np.ascontiguousarray(np.asarray(h0).astype(np.float16)[0])   # [B, H]
    y0f = np.asarray(y0).astype(np.float16)                            # [B]

    aux = _prep_aux(W_ih, W_hh, b_ih, b_hh, Wo, bo)

    if "nc" not in _NC_CACHE:
        _NC_CACHE["nc"] = build(T)
    nc = _NC_CACHE["nc"]

    in_maps = []
    for c in range(NCORES):
        sl = slice(c * BS, (c + 1) * BS)
        m = dict(aux)
        pk = aux["pack"].copy()
        pk[0, _Y00:_Y00 + 512] = y0f[sl]
        m["pack"] = pk
        m["feats"] = future_feats[sl]
        m["h0"] = h0f[sl]
        in_maps.append(m)

    res = run_bass_kernel_spmd(nc, in_maps, core_ids=list(range(NCORES)))
    outs = [r["yT"] for r in res.results]
    return np.concatenate([o.T.astype(np.float32) for o in outs], axis=0)


# revision 33
# speedup vs baseline: 1.2447x; 1.0006x over previous
"""GRU decoder kernel for Trainium2 (8 NeuronCores, data-parallel over batch).

Problem (hardcoded): B=4096, T=168, D=64, H=128.
  per step t:  gx_t = feats_t @ W_ih[:, :D].T + b_ih + y_prev * w_y
               gh   = h @ W_hh.T + b_hh
               r = sig(gx_r+gh_r); z = sig(gx_z+gh_z)
               n = tanh(gx_n + r*gh_n)
               h = (1-z)*n + z*h;  y = h @ wo + bo

Mapping per core (batch shard BS=512), layout [gate/hidden dim on
partitions, batch on free]:
  - Host pre-casts feats/h0/weights to fp16. feats stream in natural
    [b, (t,d)] layout via plain chunked DMAs, then PE transpose-mode flips
    each [128b, 128] block into [d, b] tiles (2 timesteps stacked on
    partitions: even t rows 0-63, odd rows 64-127); one ACT copy per t-pair
    evacuates PSUM -> SBUF.
  - For t>=1 the y-feedback is folded into the hidden matmuls:
      r/z:  W1 = W_hh[rz] + w_y[rz] (x) wo
      n:    rank-1 matmul (wo (x) w_y_n) @ h accumulated onto the gx_n psum
    (valid because y_prev = wo . h_prev + bo; bo is folded into biases).
    Step 0 uses the supplied y0 via K=1 matmuls.
  - Gate math: sigmoids + tanh on ACT (psum-source, per-partition biases
    ride free), the two n-gate combines as DVE scalar_tensor_tensor,
    h' = n + z*(h-n) split DVE/GpSimd. All gate tensors fp16.
  - y_t = wo . h_t via an M=32 matmul into psum col-group t%4; every 4 steps
    one DVE op (+bo, ->fp16) appends to an SBUF accumulator, DMA'd out once
    at the end (output [T, BS] fp16; host transposes + casts to fp32).
"""

import numpy as np

import concourse.bacc as bacc
import concourse.bass as bass
import concourse.mybir as mybir
import concourse.tile as tile
from concourse.bass_utils import run_bass_kernel_spmd

B, T, D, H = 4096, 168, 64, 128
NCORES = 8
BS = B // NCORES  # 512

F32 = mybir.dt.float32
F16 = mybir.dt.float16
AF = mybir.ActivationFunctionType
ALU = mybir.AluOpType

TC = 28     # timesteps per feats DMA chunk
NSLOT = 8   # featsT ring slots (t-pairs)

# packed-constants fp16 column layout
_WOC0, _WYR0, _Y00, _B0, _ID0 = 0, 32, 416, 928, 944
NPACK = 944 + 16 + 128


def build(nt=T):
    """Build the per-core Bass program. nt: number of timesteps (tests)."""
    assert nt % 4 == 0
    nc = bacc.Bacc("TRN2", target_bir_lowering=False, debug=False)

    feats = nc.declare_dram_parameter("feats", [BS, nt, D], F16, isOutput=False)
    h0 = nc.declare_dram_parameter("h0", [BS, H], F16, isOutput=False)
    wft_d = nc.declare_dram_parameter("wft", [128, 384], F16, isOutput=False)
    w1t_d = nc.declare_dram_parameter("w1t", [128, 256], F16, isOutput=False)
    whhnt_d = nc.declare_dram_parameter("whhnt", [128, 128], F16, isOutput=False)
    wynt_d = nc.declare_dram_parameter("wynt", [128, 128], F16, isOutput=False)
    whhrz0_d = nc.declare_dram_parameter("whhrz0", [128, 256], F16, isOutput=False)
    pack = nc.declare_dram_parameter("pack", [128, NPACK], F16, isOutput=False)

    yT = nc.declare_dram_parameter("yT", [nt, BS], F16, isOutput=True)
    nflush = nt // 4
    nchunk = (nt + TC - 1) // TC

    with tile.TileContext(nc) as tc:
        with (
            tc.tile_pool(name="wpool", bufs=1) as wpool,
            tc.tile_pool(name="fraw", bufs=2) as fraw_pool,
            tc.tile_pool(name="ftr", bufs=NSLOT) as ftr_pool,
            tc.tile_pool(name="hpool", bufs=2) as hpool,
            tc.tile_pool(name="gpool", bufs=2) as gpool,
            tc.tile_pool(name="ps_rz", bufs=2, space="PSUM") as ps_rz,
            tc.tile_pool(name="ps_gx", bufs=1, space="PSUM") as ps_gx,
            tc.tile_pool(name="ps_gh", bufs=1, space="PSUM") as ps_gh,
            tc.tile_pool(name="ps_u", bufs=1, space="PSUM") as ps_u,
            tc.tile_pool(name="ps_tr", bufs=1, space="PSUM") as ps_tr,
        ):
            # ---- constants ----
            pk = wpool.tile([128, NPACK], F16)
            nc.sync.dma_start(pk[:], pack[:])
            woc = pk[:, _WOC0:_WOC0 + 32]
            wyrow = pk[0:1, _WYR0:_WYR0 + 384]
            y0_sb = pk[0:1, _Y00:_Y00 + 512]
            brz1 = pk[:, _B0:_B0 + 4].bitcast(F32)
            brz0 = pk[:, _B0 + 4:_B0 + 8].bitcast(F32)
            bn1 = pk[:, _B0 + 8:_B0 + 10].bitcast(F32)
            bn0 = pk[:, _B0 + 10:_B0 + 12].bitcast(F32)
            bhn = pk[:, _B0 + 12:_B0 + 14].bitcast(F32)
            bo_t = pk[:, _B0 + 14:_B0 + 16].bitcast(F32)
            ident = pk[:, _ID0:_ID0 + 128]

            wft = wpool.tile([128, 384], F16)
            w1t = wpool.tile([128, 256], F16)
            whhnt = wpool.tile([128, 128], F16)
            wynt = wpool.tile([128, 128], F16)
            whhrz0 = wpool.tile([128, 256], F16)
            for sb, dr in [
                (wft, wft_d), (w1t, w1t_d), (whhnt, whhnt_d),
                (wynt, wynt_d), (whhrz0, whhrz0_d),
            ]:
                nc.sync.dma_start(sb[:], dr[:])

            # ---- h0 -> hT via PE transpose ----
            h0st = wpool.tile([128, 512], F16)
            for j in range(4):
                nc.sync.dma_start(
                    h0st[:, j * 128:(j + 1) * 128], h0[j * 128:(j + 1) * 128, :],
                )
            p_t0 = ps_tr.tile([128, BS], F16, tag="p_tr")
            for j in range(4):
                nc.tensor.transpose(
                    p_t0[:, j * 128:(j + 1) * 128], h0st[:, j * 128:(j + 1) * 128],
                    ident,
                )
            hT_prev = hpool.tile([128, BS], F16, tag="hT")
            nc.scalar.copy(hT_prev[:], p_t0[:])

            # ---- feats pipeline ----
            ftr_tiles = {}
            raw_tiles = {}

            def load_chunk(c):
                t0 = c * TC
                tcn = min(TC, nt - t0)
                raws = []
                for j in range(4):
                    raw = fraw_pool.tile([128, TC * 64], F16, tag=f"fraw{j}")
                    nc.sync.dma_start(
                        raw[:, : tcn * 64],
                        feats[j * 128:(j + 1) * 128, t0:t0 + tcn, :],
                    )
                    raws.append(raw)
                raw_tiles[c] = raws

            def transpose_pair(p):
                c, po = divmod(p, TC // 2)
                raws = raw_tiles[c]
                p_tr = ps_tr.tile([128, BS], F16, tag="p_tr")
                for j in range(4):
                    nc.tensor.transpose(
                        p_tr[:, j * 128:(j + 1) * 128],
                        raws[j][:, po * 128:(po + 1) * 128], ident,
                    )
                ftile = ftr_pool.tile([128, BS], F16, tag="ftr")
                nc.scalar.copy(ftile[:], p_tr[:])
                ftr_tiles[p] = ftile

            load_chunk(0)
            yacc = wpool.tile([128, nflush * 512], F16)

            # ---- recurrence ----
            # feats matmuls are emitted one step ahead of the serial chain so
            # they fill PE gaps while the previous step's gate math runs.
            psums = {}

            def alloc_feats_pair(t):
                # r/z feats matmuls for the (even, odd) timestep pair run
                # concurrently on disjoint PE row-groups (rows 0-63 = even t,
                # 64-127 = odd t in both wft and the ftile).
                ftile = ftr_tiles[t // 2]
                pe = {}
                for half, tt in ((0, t), (64, t + 1)):
                    p_r = ps_rz.tile([128, BS], F32, tag="p_r")
                    p_z = ps_rz.tile([128, BS], F32, tag="p_z")
                    rhs_f = ftile[half:half + 64, :]
                    wf = wft[half:half + 64, :]
                    nc.tensor.matmul(p_r[:], wf[:, 0:128], rhs_f,
                                     start=True, stop=False, tile_position=(half, 0))
                    nc.tensor.matmul(p_z[:], wf[:, 128:256], rhs_f,
                                     start=True, stop=False, tile_position=(half, 0))
                    pe[tt] = (p_r, p_z)
                return pe

            gxs = {}

            def alloc_gx(t):
                ftile = ftr_tiles[t // 2]
                half = (t % 2) * 64
                p_gx = ps_gx.tile([128, BS], F32, tag="p_gx")
                nc.tensor.matmul(p_gx[:], wft[half:half + 64, 256:384],
                                 ftile[half:half + 64, :], start=True, stop=False)
                gxs[t] = p_gx

            for t in range(nt):
                if t % TC == 0 and t // TC + 1 < nchunk:
                    load_chunk(t // TC + 1)
                if t % 2 == 0:
                    # stay a few pairs ahead of the consumer
                    want = min(t // 2 + 2, nt // 2 - 1)
                    for p in range(len(ftr_tiles), want + 1):
                        transpose_pair(p)
                if t == 0:
                    psums.update(alloc_feats_pair(0))
                    alloc_gx(0)

                p_r, p_z = psums.pop(t)
                p_gx = gxs.pop(t)
                p_gh = ps_gh.tile([128, BS], F32, tag="p_gh")

                # hidden matmuls — r-path and gh first (they feed the chain)
                if t == 0:
                    nc.tensor.matmul(p_r[:], whhrz0[:, 0:128], hT_prev[:], start=False, stop=False)
                    nc.tensor.matmul(p_r[:], wyrow[0:1, 0:128], y0_sb, start=False, stop=True)
                    nc.tensor.matmul(p_gh[:], whhnt[:], hT_prev[:], start=True, stop=True)
                    nc.tensor.matmul(p_gx[:], wyrow[0:1, 256:384], y0_sb, start=False, stop=True)
                    nc.tensor.matmul(p_z[:], whhrz0[:, 128:256], hT_prev[:], start=False, stop=False)
                    nc.tensor.matmul(p_z[:], wyrow[0:1, 128:256], y0_sb, start=False, stop=True)
                else:
                    nc.tensor.matmul(p_r[:], w1t[:, 0:128], hT_prev[:], start=False, stop=True)
                    nc.tensor.matmul(p_gh[:], whhnt[:], hT_prev[:], start=True, stop=True)
                    nc.tensor.matmul(p_gx[:], wynt[:], hT_prev[:], start=False, stop=True)
                    nc.tensor.matmul(p_z[:], w1t[:, 128:256], hT_prev[:], start=False, stop=True)
                if t + 1 < nt:
                    alloc_gx(t + 1)
                    if (t + 1) % 2 == 0:
                        psums.update(alloc_feats_pair(t + 1))

                brz = brz0 if t == 0 else brz1
                bn = bn0 if t == 0 else bn1

                r16 = gpool.tile([128, BS], F16, tag="r16")
                z16 = gpool.tile([128, BS], F16, tag="z16")
                nc.scalar.activation(r16[:], p_r[:], AF.Sigmoid, bias=brz[:, 0:1])
                nc.scalar.activation(z16[:], p_z[:], AF.Sigmoid, bias=brz[:, 1:2])

                t1 = gpool.tile([128, BS], F16, tag="t1")
                npre = gpool.tile([128, BS], F16, tag="npre")
                n16 = gpool.tile([128, BS], F16, tag="n16")
                # t1 = (gh_n + b_hh_n) * r ; npre = (gx_n + b_n) + t1
                nc.vector.scalar_tensor_tensor(t1[:], p_gh[:], bhn[:, 0:1], r16[:], ALU.add, ALU.mult)
                nc.vector.scalar_tensor_tensor(npre[:], p_gx[:], bn[:, 0:1], t1[:], ALU.add, ALU.add)
                nc.scalar.activation(n16[:], npre[:], AF.Tanh)

                hmn = gpool.tile([128, BS], F16, tag="hmn")
                zh = gpool.tile([128, BS], F16, tag="zh")
                hT_cur = hpool.tile([128, BS], F16, tag="hT")
                nc.vector.tensor_tensor(hmn[:], hT_prev[:], n16[:], ALU.subtract)
                nc.vector.tensor_tensor(zh[:], z16[:], hmn[:], ALU.mult)
                nc.vector.tensor_tensor(hT_cur[:], n16[:], zh[:], ALU.add)

                # y_t = wo . h_t (+bo), 32 duplicated rows per psum col-group
                c4 = t % 4
                if c4 == 0:
                    p_u = ps_u.tile([128, BS], F32, tag="p_u")
                nc.tensor.matmul(
                    p_u[32 * c4:32 * (c4 + 1), :], woc, hT_cur[:],
                    start=True, stop=True, tile_position=(0, 32 * c4),
                )
                if c4 == 3:
                    g = t // 4
                    nc.vector.tensor_scalar(
                        yacc[:, g * 512:(g + 1) * 512], p_u[:], bo_t[:, 0:1],
                        None, ALU.add,
                    )

                hT_prev = hT_cur

            # ---- single y writeback ----
            nc.sync.dma_start(
                yT.rearrange("(g c) b -> c g b", c=4),
                yacc[0:128:32, :].rearrange("p (g b) -> p g b", b=512),
            )

    nc.compile()
    return nc


# -------- host-side weight prep + sharded execution --------

def _prep_aux(W_ih, W_hh, b_ih, b_hh, Wo, bo):
    W_ih = np.asarray(W_ih, np.float32)
    W_hh = np.asarray(W_hh, np.float32)
    b_ih = np.asarray(b_ih, np.float32)
    b_hh = np.asarray(b_hh, np.float32)
    wo = np.asarray(Wo, np.float32)[0]       # [H]
    bo_s = float(np.asarray(bo, np.float32)[0])
    wfd = W_ih[:, :D]                         # [3H, D]
    w_y = W_ih[:, D]                          # [3H]

    wft = np.zeros((128, 384), np.float16)
    wft[0:64] = wfd.T.astype(np.float16)
    wft[64:128] = wfd.T.astype(np.float16)

    W1 = W_hh[0:2 * H] + np.outer(w_y[0:2 * H], wo)       # [2H, H]
    aux = dict(
        wft=wft,
        w1t=np.ascontiguousarray(W1.T.astype(np.float16)),
        whhnt=np.ascontiguousarray(W_hh[2 * H:].T.astype(np.float16)),
        wynt=np.ascontiguousarray(np.outer(wo, w_y[2 * H:]).astype(np.float16)),
        whhrz0=np.ascontiguousarray(W_hh[0:2 * H].T.astype(np.float16)),
    )

    pk = np.zeros((128, NPACK), np.float16)
    pk[:, _WOC0:_WOC0 + 32] = np.repeat(wo[:, None], 32, axis=1).astype(np.float16)
    pk[0, _WYR0:_WYR0 + 384] = w_y.astype(np.float16)
    brz_base = (b_ih + b_hh)[0:2 * H]
    brz1 = np.stack(
        [brz_base[0:H] + w_y[0:H] * bo_s, brz_base[H:2 * H] + w_y[H:2 * H] * bo_s],
        axis=1,
    ).astype(np.float32)
    brz0 = np.stack([brz_base[0:H], brz_base[H:2 * H]], axis=1).astype(np.float32)
    bn1 = (b_ih[2 * H:] + w_y[2 * H:] * bo_s)[:, None].astype(np.float32)
    bn0 = b_ih[2 * H:][:, None].astype(np.float32)
    bhn = b_hh[2 * H:][:, None].astype(np.float32)
    bo_a = np.full((128, 1), bo_s, np.float32)
    block = np.concatenate([brz1, brz0, bn1, bn0, bhn, bo_a], axis=1)  # [128, 8]
    pk[:, _B0:_B0 + 16] = np.ascontiguousarray(block).view(np.float16)
    pk[:, _ID0:_ID0 + 128] = np.eye(128, dtype=np.float16)
    aux["pack"] = pk
    return aux


_NC_CACHE = {}


def kernel(future_feats, h0, y0, W_ih, W_hh, b_ih, b_hh, Wo, bo):
    future_feats = np.ascontiguousarray(np.asarray(future_feats).astype(np.float16))
    h0f = np.ascontiguousarray(np.asarray(h0).astype(np.float16)[0])   # [B, H]
    y0f = np.asarray(y0).astype(np.float16)                            # [B]

    aux = _prep_aux(W_ih, W_hh, b_ih, b_hh, Wo, bo)

    if "nc" not in _NC_CACHE:
        _NC_CACHE["nc"] = build(T)
    nc = _NC_CACHE["nc"]

    in_maps = []
    for c in range(NCORES):
        sl = slice(c * BS, (c + 1) * BS)
        m = dict(aux)
        pk = aux["pack"].copy()
        pk[0, _Y00:_Y00 + 512] = y0f[sl]
        m["pack"] = pk
        m["feats"] = future_feats[sl]
        m["h0"] = h0f[sl]
        in_maps.append(m)

    res = run_bass_kernel_spmd(nc, in_maps, core_ids=list(range(NCORES)))
    outs = [r["yT"] for r in res.results]
    return np.concatenate([o.T.astype(np.float32) for o in outs], axis=0)
